# revision 11
# baseline (speedup 1.0000x reference)
"""Trainium2 Bass kernel for nn_InvariantCrossAttention.

Math: the reference computes softmax(-(Q2_i + K2_j), axis=j) — but -Q2_i is
constant along the softmax axis, so it cancels. The attention row is the same
for every query i, hence context[b,i] is i-independent and the final mean over
N is a no-op:

    out[b] = sum_j exp(-K2[b,j]) * K2[b,j] / sum_j exp(-K2[b,j])
    K2[b,j] = (x[b,j] - mean_j x[b,:])^2,  x = all_atom_features[:, :, 0]

cdr3_features does not affect the output (for any input values). The kernel
computes the reduction above on-device. Sharding: the post-simplification
problem is 128KB of input and ~20 instructions, so every core runs the full
(replicated) computation and core 0's output is returned — any cross-core
split would put a collective (multi-us) on a sub-us critical path.

Layout: x viewed as [128 partitions, 256 cols]; partition p holds batch p//32
(32 partitions per batch, contiguous 1KB rows -> full DMA bandwidth).
Cross-partition per-batch reduce/broadcast are tiny PE matmuls against
memset-generated group masks. The input load is split across both HWDGE
rings (SP + Activation) so the two halves' completion latencies overlap, and
the per-batch sum consumes each half directly via PSUM accumulation.
"""

import os

import numpy as np

B = 4  # batch
M = 8192  # all_atom length (softmax axis)
P = 128  # SBUF partitions
COLS = B * M // P  # 256 elements per partition
PPB = P // B  # 32 partitions per batch
N_CORES = 8

_cache = {}
last_results = None  # BassKernelResults of the most recent run (for test.py)


def _build():
    import concourse.bacc as bacc
    import concourse.bass as bass
    import concourse.mybir as mybir
    import concourse.tile as tile

    f32 = mybir.dt.float32
    nc = bacc.Bacc("TRN2", target_bir_lowering=False, debug=False)

    x_dram = nc.dram_tensor("x", [P, COLS], f32, kind="ExternalInput")
    nmaskT_dram = nc.dram_tensor("nmaskT", [B, P], f32, kind="ExternalInput")
    out_dram = nc.dram_tensor("out", [B, 1], f32, kind="ExternalOutput")

    with tile.TileContext(nc) as tc:
        with (
            tc.tile_pool(name="sbuf", bufs=1) as pool,
            tc.tile_pool(name="psum", bufs=1, space=bass.MemorySpace.PSUM) as psum,
        ):
            X = pool.tile([P, COLS], f32)
            mask = pool.tile([P, B], f32)
            nmaskT = pool.tile([B, P], f32)
            zb = pool.tile([P, 1], f32)

            # Group masks built by memsets (off the critical path; no DMA).
            # mask[p,b] = 1 iff p//32 == b; nmaskT[b,p] = -1/M iff p//32 == b.
            nc.vector.memset(mask[:], 0.0)
            for b in range(B):
                nc.vector.memset(mask[b * PPB : (b + 1) * PPB, b : b + 1], 1.0)
            nc.gpsimd.memset(zb[:], 0.0)

            # Dummy activation so the ACT table load (~1.3us) overlaps the
            # input DMA instead of stalling the first real ACTIVATE.
            warm = pool.tile([1, 1], f32)
            nc.scalar.activation(
                warm[:],
                zb[0:1, 0:1],
                mybir.ActivationFunctionType.Exp,
                bias=zb[0:1, 0:1],
                scale=-1.0,
            )

            # Input halves on the two HWDGE rings (SP and Activation) so the
            # completion-receipt latencies overlap.
            H = P // 2
            nc.sync.dma_start(X[0:H, :], x_dram[0:H, :])
            nc.scalar.dma_start(X[H:P, :], x_dram[H:P, :])
            # nmaskT (partition offsets 1..3 are not quadrant-aligned, so it
            # can't be memset-built) rides the Scalar ring behind X_h1; its
            # completion lands well before the broadcast matmul needs it.
            nc.scalar.dma_start(nmaskT[:], nmaskT_dram[:])

            # Per-batch sums: accumulate mask.T @ X over the two halves into
            # PSUM [4, 256], then one free-axis reduce -> s4 [4,1] in SBUF.
            SX = psum.tile([B, COLS], f32)
            nc.tensor.matmul(SX[:], mask[:], X[:])
            s4 = pool.tile([B, 1], f32)
            nc.vector.reduce_sum(s4[:], SX[:], axis=mybir.AxisListType.X)

            # Broadcast negative means to [128,1]: nmaskT.T @ s4.
            NM = psum.tile([P, 1], f32)
            nc.tensor.matmul(NM[:], nmaskT[:], s4[:])
            nm = pool.tile([P, 1], f32)
            nc.vector.tensor_copy(nm[:], NM[:])

            # K2 = (x - mean)^2; w = exp(-K2) with per-partition sum;
            # wk = w*K2 with per-partition sum; mask.T @ [s1|s2] -> [4,2].
            K2 = pool.tile([P, COLS], f32)
            nc.scalar.activation(
                K2[:], X[:], mybir.ActivationFunctionType.Square, bias=nm[:]
            )

            partials = pool.tile([P, 2], f32)
            w = pool.tile([P, COLS], f32)
            nc.scalar.activation(
                w[:],
                K2[:],
                mybir.ActivationFunctionType.Exp,
                bias=zb[:],
                scale=-1.0,
                accum_out=partials[:, 0:1],
            )

            wk = pool.tile([P, COLS], f32)
            nc.vector.scalar_tensor_tensor(
                wk[:],
                w[:],
                1.0,
                K2[:],
                op0=mybir.AluOpType.mult,
                op1=mybir.AluOpType.mult,
                accum_out=partials[:, 1:2],
            )

            S2 = psum.tile([B, 2], f32)
            nc.tensor.matmul(
                S2[:], mask[:], partials[:]
            )

            r = pool.tile([B, 1], f32)
            nc.vector.reciprocal(r[:], S2[:, 0:1])
            res = pool.tile([B, 1], f32)
            nc.vector.tensor_tensor(
                res[:], S2[:, 1:2], r[:], op=mybir.AluOpType.mult
            )

            nc.sync.dma_start(out_dram[:], res[:])

    nc.compile()
    return nc


def kernel(cdr3_features=None, all_atom_features=None, **_unused):
    from concourse.bass_utils import run_bass_kernel_spmd

    global last_results
    if "nc" not in _cache:
        _cache["nc"] = _build()
    nc = _cache["nc"]

    x = np.ascontiguousarray(np.asarray(all_atom_features, dtype=np.float32)).reshape(
        P, COLS
    )
    nmaskT = np.zeros((B, P), np.float32)
    for b in range(B):
        nmaskT[b, b * PPB : (b + 1) * PPB] = np.float32(-1.0 / M)
    in_map = {"x": x, "nmaskT": nmaskT}

    trace = bool(os.environ.get("KERNEL_TRACE"))
    last_results = run_bass_kernel_spmd(
        nc, [in_map] * N_CORES, list(range(N_CORES)), trace=trace
    )
    out = np.asarray(last_results.results[0]["out"], dtype=np.float32)
    return out.reshape(B, 1)


# revision 12
# speedup vs baseline: 1.0179x; 1.0179x over previous
"""Trainium2 Bass kernel for nn_InvariantCrossAttention.

Math: the reference computes softmax(-(Q2_i + K2_j), axis=j) — but -Q2_i is
constant along the softmax axis, so it cancels. The attention row is the same
for every query i, hence context[b,i] is i-independent and the final mean over
N is a no-op:

    out[b] = sum_j exp(-K2[b,j]) * K2[b,j] / sum_j exp(-K2[b,j])
    K2[b,j] = (x[b,j] - mean_j x[b,:])^2,  x = all_atom_features[:, :, 0]

cdr3_features does not affect the output (for any input values). The kernel
computes the reduction above on-device. Sharding: the post-simplification
problem is 128KB of input and ~20 instructions, so every core runs the full
(replicated) computation and core 0's output is returned — any cross-core
split would put a collective (multi-us) on a sub-us critical path.

Layout: x viewed as [128 partitions, 256 cols]; partition p holds batch p//32
(32 partitions per batch, contiguous 1KB rows -> full DMA bandwidth).
Cross-partition per-batch reduce/broadcast are tiny PE matmuls against
memset-generated group masks. The input load is split across both HWDGE
rings (SP + Activation) so the two halves' completion latencies overlap, and
the per-batch sum consumes each half directly via PSUM accumulation.
"""

import os

import numpy as np

B = 4  # batch
M = 8192  # all_atom length (softmax axis)
P = 128  # SBUF partitions
COLS = B * M // P  # 256 elements per partition
PPB = P // B  # 32 partitions per batch
N_CORES = 8

_cache = {}
last_results = None  # BassKernelResults of the most recent run (for test.py)


def _build():
    import concourse.bacc as bacc
    import concourse.bass as bass
    import concourse.mybir as mybir
    import concourse.tile as tile

    f32 = mybir.dt.float32
    nc = bacc.Bacc("TRN2", target_bir_lowering=False, debug=False)

    x_dram = nc.dram_tensor("x", [P, COLS], f32, kind="ExternalInput")
    nmaskT_dram = nc.dram_tensor("nmaskT", [B, P], f32, kind="ExternalInput")
    out_dram = nc.dram_tensor("out", [B, 1], f32, kind="ExternalOutput")

    with tile.TileContext(nc) as tc:
        with (
            tc.tile_pool(name="sbuf", bufs=1) as pool,
            tc.tile_pool(name="psum", bufs=1, space=bass.MemorySpace.PSUM) as psum,
        ):
            X = pool.tile([P, COLS], f32)
            mask = pool.tile([P, B], f32)
            nmaskT = pool.tile([B, P], f32)
            zb = pool.tile([P, 1], f32)

            # Group masks built by memsets (off the critical path; no DMA).
            # mask[p,b] = 1 iff p//32 == b; nmaskT[b,p] = -1/M iff p//32 == b.
            nc.vector.memset(mask[:], 0.0)
            for b in range(B):
                nc.vector.memset(mask[b * PPB : (b + 1) * PPB, b : b + 1], 1.0)
            nc.gpsimd.memset(zb[:], 0.0)

            # Dummy activation so the ACT table load (~1.3us) overlaps the
            # input DMA instead of stalling the first real ACTIVATE.
            warm = pool.tile([1, 1], f32)
            nc.scalar.activation(
                warm[:],
                zb[0:1, 0:1],
                mybir.ActivationFunctionType.Exp,
                bias=zb[0:1, 0:1],
                scale=-1.0,
            )

            # Input in 4 chunks alternating between the two HWDGE rings (SP
            # and Activation): chunk completion sems land progressively and
            # each chunk's partial reduce overlaps the next chunk's receipt.
            NCHUNK = 4
            CH = P // NCHUNK
            partial = pool.tile([P, 1], f32)
            for c in range(NCHUNK):
                sl = slice(c * CH, (c + 1) * CH)
                eng = nc.sync if c % 2 == 0 else nc.scalar
                eng.dma_start(X[sl, :], x_dram[sl, :])
            # nmaskT (partition offsets 1..3 are not quadrant-aligned, so it
            # can't be memset-built) rides the Scalar ring behind the X
            # chunks; its completion lands before the broadcast matmul.
            nc.scalar.dma_start(nmaskT[:], nmaskT_dram[:])
            for c in range(NCHUNK):
                sl = slice(c * CH, (c + 1) * CH)
                nc.vector.reduce_sum(
                    partial[sl, :], X[sl, :], axis=mybir.AxisListType.X
                )

            # Per-batch sums then negative-mean broadcast via tiny PE matmuls.
            S1 = psum.tile([B, 1], f32)
            nc.tensor.matmul(S1[:], mask[:], partial[:])
            s4 = pool.tile([B, 1], f32)
            nc.vector.tensor_copy(s4[:], S1[:])
            NM = psum.tile([P, 1], f32)
            nc.tensor.matmul(NM[:], nmaskT[:], s4[:])
            nm = pool.tile([P, 1], f32)
            nc.vector.tensor_copy(nm[:], NM[:])

            # K2 = (x - mean)^2; w = exp(-K2) with per-partition sum;
            # wk = w*K2 with per-partition sum; mask.T @ [s1|s2] -> [4,2].
            K2 = pool.tile([P, COLS], f32)
            nc.scalar.activation(
                K2[:], X[:], mybir.ActivationFunctionType.Square, bias=nm[:]
            )

            partials = pool.tile([P, 2], f32)
            w = pool.tile([P, COLS], f32)
            nc.scalar.activation(
                w[:],
                K2[:],
                mybir.ActivationFunctionType.Exp,
                bias=zb[:],
                scale=-1.0,
                accum_out=partials[:, 0:1],
            )

            wk = pool.tile([P, COLS], f32)
            nc.vector.scalar_tensor_tensor(
                wk[:],
                w[:],
                1.0,
                K2[:],
                op0=mybir.AluOpType.mult,
                op1=mybir.AluOpType.mult,
                accum_out=partials[:, 1:2],
            )

            S2 = psum.tile([B, 2], f32)
            nc.tensor.matmul(
                S2[:], mask[:], partials[:]
            )

            r = pool.tile([B, 1], f32)
            nc.vector.reciprocal(r[:], S2[:, 0:1])
            res = pool.tile([B, 1], f32)
            nc.vector.tensor_tensor(
                res[:], S2[:, 1:2], r[:], op=mybir.AluOpType.mult
            )

            nc.sync.dma_start(out_dram[:], res[:])

    nc.compile()
    return nc


def kernel(cdr3_features=None, all_atom_features=None, **_unused):
    from concourse.bass_utils import run_bass_kernel_spmd

    global last_results
    if "nc" not in _cache:
        _cache["nc"] = _build()
    nc = _cache["nc"]

    x = np.ascontiguousarray(np.asarray(all_atom_features, dtype=np.float32)).reshape(
        P, COLS
    )
    nmaskT = np.zeros((B, P), np.float32)
    for b in range(B):
        nmaskT[b, b * PPB : (b + 1) * PPB] = np.float32(-1.0 / M)
    in_map = {"x": x, "nmaskT": nmaskT}

    trace = bool(os.environ.get("KERNEL_TRACE"))
    last_results = run_bass_kernel_spmd(
        nc, [in_map] * N_CORES, list(range(N_CORES)), trace=trace
    )
    out = np.asarray(last_results.results[0]["out"], dtype=np.float32)
    return out.reshape(B, 1)
